# revision 7
# baseline (speedup 1.0000x reference)
"""Trainium2 Bass kernel for 12-head causal MHA (B=4, S=2048, D=768).

Sharding: 8 cores, core c -> (batch c//2, query-row parity c%2).
Each core computes the full attention output for query rows
g = 2*t + parity of its batch (1024 rows), which makes the causal loop
structure identical on every core (single SPMD Bass program) and the
gather a pure row-interleave.

Layout is fully transposed so every matmul contracts along partitions:
  qT/kT: [head_dim, seq]  scoresT: [sk, sq]  ctxT': [hd+1, sq]
The softmax row-sum is fused into the ctx matmul via a ones column
appended to V (M=65).  Softmax skips max-subtraction (scores/8 are
bounded by ~2 for this distribution, exp is safe).
"""

import os
import sys
from contextlib import ExitStack

import numpy as np

os.environ.setdefault("MYCRO_LOCAL_CACHE", "1")

for _p in ("/root/.axon_site/_ro/trn_rl_repo", "/opt/trn_rl_repo"):
    # later inserts win: prefer /opt (writable sibling modules, e.g.
    # antenv.axon_hooks) over the read-only mirror
    if os.path.isdir(_p) and _p not in sys.path:
        sys.path.insert(0, _p)

import concourse.bass as bass  # noqa: E402
import concourse.tile as tile  # noqa: E402
from concourse import bacc, mybir  # noqa: E402
from concourse.bass_utils import run_bass_kernel_spmd  # noqa: E402

B, S, D, H, HD = 4, 2048, 768, 12, 64
NPAIR = H // 2          # 6 head pairs (2 heads packed per 128 partitions)
SQL = S // 2            # 1024 local query rows per core
JB = SQL // 256         # 4 local 256-col blocks
KC = S // 128           # 16 key chunks
DC = D // 128           # 6 contraction chunks for the projections
N_CORES = 8

F32 = mybir.dt.float32
F32R = mybir.dt.float32r
EXP = mybir.ActivationFunctionType.Exp

LAST_RESULT = None  # BassKernelResults of the most recent run (for test.py)

_CACHED_NC = None


def _r(ap):
    """f32r matmul operand (tiles on these paths are float32r-typed)."""
    return ap


def build_nc():
    nc = bacc.Bacc("TRN2", target_bir_lowering=False)

    xT = nc.dram_tensor("xT", [D, S], F32R, kind="ExternalInput")
    xTq = nc.dram_tensor("xTq", [D, SQL], F32R, kind="ExternalInput")
    wqT = nc.dram_tensor("wqT", [D, D], F32R, kind="ExternalInput")
    wkT = nc.dram_tensor("wkT", [D, D], F32R, kind="ExternalInput")
    wvT = nc.dram_tensor("wvT", [D, D], F32R, kind="ExternalInput")
    woT = nc.dram_tensor("woT", [D, D], F32R, kind="ExternalInput")
    masks = nc.dram_tensor("masks", [4, 128, 256], F32R, kind="ExternalInput")
    bo_d = nc.dram_tensor("bo", [1, D], F32, kind="ExternalInput")
    out_d = nc.dram_tensor("out", [SQL, D], F32, kind="ExternalOutput")

    with tile.TileContext(nc) as tc, ExitStack() as ctx:
        pers = ctx.enter_context(tc.tile_pool(name="pers", bufs=1))
        kT6 = pers.tile([128, NPAIR, S], F32R)          # kT, pair-stacked
        v3 = pers.tile([128, KC, H, HD + 1], F32R)      # v (+ones col) per chunk
        qT6 = pers.tile([128, NPAIR, SQL], F32R)
        ctx6 = pers.tile([128, NPAIR, SQL], F32R)       # normalized ctxT
        ones_sb = pers.tile([65, 128], F32)
        mask_sb = pers.tile([128, 4, 256], F32R)
        bo_sb = pers.tile([128, D], F32)

        nc.vector.memset(ones_sb, 1.0)
        # ones cols, stride 65 (f32 view: walrus rejects f32r memsets)
        nc.vector.memset(v3.bitcast(F32)[:, :, :, HD], 1.0)
        for mi in range(4):
            nc.sync.dma_start(out=mask_sb[:, mi, :], in_=masks[mi])

        # --- broadcast bo across partitions once (rank-1 matmul trick) ---
        with (
            tc.tile_pool(name="pre_s", bufs=1) as pre_s,
            tc.tile_pool(name="pre_p", bufs=1, space="PSUM") as pre_p,
        ):
            bo_row = pre_s.tile([1, D], F32)
            nc.sync.dma_start(out=bo_row, in_=bo_d[:])
            pbo = pre_p.tile([128, D], F32)
            for lo, hi in ((0, 512), (512, D)):
                nc.tensor.matmul(pbo[:, lo:hi], lhsT=ones_sb[0:1, :],
                                 rhs=bo_row[0:1, lo:hi], start=True, stop=True)
            nc.vector.tensor_copy(bo_sb, pbo)

        # --- projections: K, V, then Q (weights staged one at a time) ---
        with (
            tc.tile_pool(name="wstage", bufs=1) as wpool,
            tc.tile_pool(name="xstage", bufs=2) as xpool,
            tc.tile_pool(name="pproj", bufs=2, space="PSUM") as ppool,
        ):
            # K projection: kT6[:, r, s] for all 2048 keys
            wk = wpool.tile([128, DC, D], F32R, tag="w")
            for k in range(DC):
                nc.sync.dma_start(out=wk[:, k, :], in_=wkT[128 * k:128 * (k + 1), :])
            for sb in range(S // 256):
                xk = xpool.tile([128, DC, 256], F32R, tag="x")
                for k in range(DC):
                    nc.sync.dma_start(
                        out=xk[:, k, :],
                        in_=xT[128 * k:128 * (k + 1), 256 * sb:256 * (sb + 1)])
                for r in range(NPAIR):
                    ps = ppool.tile([128, 256], F32, tag="pk")
                    for k in range(DC):
                        nc.tensor.matmul(
                            ps, lhsT=_r(wk[:, k, 128 * r:128 * (r + 1)]),
                            rhs=_r(xk[:, k, :]),
                            start=(k == 0), stop=(k == DC - 1))
                    nc.vector.tensor_copy(kT6[:, r, 256 * sb:256 * (sb + 1)], ps)

            # V projection: v3[:, a, h, 0:64] per 128-key chunk a
            wv = wpool.tile([128, DC, D], F32R, tag="w")
            for k in range(DC):
                nc.sync.dma_start(out=wv[:, k, :], in_=wvT[128 * k:128 * (k + 1), :])
            for a in range(KC):
                xa = xpool.tile([128, DC, 128], F32R, tag="x")
                for k in range(DC):
                    nc.sync.dma_start(
                        out=xa[:, k, :],
                        in_=xT[128 * k:128 * (k + 1), 128 * a:128 * (a + 1)])
                ps = ppool.tile([128, D], F32, tag="pv")
                for lo, hi in ((0, 512), (512, D)):
                    for k in range(DC):
                        nc.tensor.matmul(
                            ps[:, lo:hi], lhsT=_r(xa[:, k, :]),
                            rhs=_r(wv[:, k, lo:hi]),
                            start=(k == 0), stop=(k == DC - 1))
                nc.vector.tensor_copy(
                    v3[:, a, :, 0:HD],
                    ps.rearrange("p (h e) -> p h e", e=HD))

            # Q projection (only this core's 1024 query rows)
            wq = wpool.tile([128, DC, D], F32R, tag="w")
            for k in range(DC):
                nc.sync.dma_start(out=wq[:, k, :], in_=wqT[128 * k:128 * (k + 1), :])
            for j in range(JB):
                xq = xpool.tile([128, DC, 256], F32R, tag="x")
                for k in range(DC):
                    nc.sync.dma_start(
                        out=xq[:, k, :],
                        in_=xTq[128 * k:128 * (k + 1), 256 * j:256 * (j + 1)])
                for r in range(NPAIR):
                    ps = ppool.tile([128, 256], F32, tag="pk")
                    for k in range(DC):
                        nc.tensor.matmul(
                            ps, lhsT=_r(wq[:, k, 128 * r:128 * (r + 1)]),
                            rhs=_r(xq[:, k, :]),
                            start=(k == 0), stop=(k == DC - 1))
                    nc.vector.tensor_copy(qT6[:, r, 256 * j:256 * (j + 1)], ps)

        # --- attention ---
        with (
            tc.tile_pool(name="spool", bufs=4, space="PSUM") as spool,
            tc.tile_pool(name="cpool", bufs=2, space="PSUM") as cpool,
            tc.tile_pool(name="bpool", bufs=1, space="PSUM") as bpool,
            tc.tile_pool(name="epool", bufs=3) as epool,
            tc.tile_pool(name="rpool", bufs=2) as rpool,
        ):
            for r in range(NPAIR):
                for j in range(JB):
                    nch = 4 * j + 4
                    cA = cpool.tile([65, 256], F32, tag="c")
                    cB = cpool.tile([65, 256], F32, tag="c")
                    jsl = slice(256 * j, 256 * (j + 1))
                    for a in range(nch):
                        asl = slice(128 * a, 128 * (a + 1))
                        sA = spool.tile([128, 256], F32, tag="s")
                        sB = spool.tile([128, 256], F32, tag="s")
                        nc.tensor.matmul(
                            sA, lhsT=_r(kT6[0:64, r, asl]),
                            rhs=_r(qT6[0:64, r, jsl]),
                            start=True, stop=True, tile_position=(0, 0))
                        nc.tensor.matmul(
                            sB, lhsT=_r(kT6[64:128, r, asl]),
                            rhs=_r(qT6[64:128, r, jsl]),
                            start=True, stop=True, tile_position=(64, 0))
                        e = epool.tile([128, 512], F32R, tag="e")
                        nc.scalar.activation(e[:, 0:256], sA, EXP, scale=0.125)
                        nc.scalar.activation(e[:, 256:512], sB, EXP, scale=0.125)
                        mi = a - 4 * j
                        if mi >= 0:
                            nc.vector.tensor_mul(
                                e[:, 0:256], e[:, 0:256], mask_sb[:, mi, :])
                            nc.vector.tensor_mul(
                                e[:, 256:512], e[:, 256:512], mask_sb[:, mi, :])
                        nc.tensor.matmul(
                            cA, lhsT=_r(v3[:, a, 2 * r, :]), rhs=_r(e[:, 0:256]),
                            start=(a == 0), stop=(a == nch - 1))
                        nc.tensor.matmul(
                            cB, lhsT=_r(v3[:, a, 2 * r + 1, :]),
                            rhs=_r(e[:, 256:512]),
                            start=(a == 0), stop=(a == nch - 1))
                    # normalize: recip of fused row-sums, broadcast via PE
                    rr = rpool.tile([65, 512], F32, tag="rr")
                    nc.vector.reciprocal(rr[64:65, 0:256], cA[64:65, :])
                    nc.vector.reciprocal(rr[64:65, 256:512], cB[64:65, :])
                    pb = bpool.tile([128, 512], F32, tag="b")
                    nc.tensor.matmul(pb, lhsT=ones_sb[64:65, :],
                                     rhs=rr[64:65, :], start=True, stop=True)
                    pb_sb = rpool.tile([128, 512], F32, tag="pbs")
                    nc.vector.tensor_copy(pb_sb, pb)
                    nc.vector.tensor_mul(ctx6[0:64, r, jsl], cA[0:64, :],
                                         pb_sb[0:64, 0:256])
                    tB = rpool.tile([64, 256], F32R, tag="tB")
                    nc.vector.tensor_mul(tB, cB[0:64, :], pb_sb[0:64, 256:512])
                    # head B lands on partitions 64-127: remap via SBUF DMA
                    nc.sync.dma_start(out=ctx6[64:128, r, jsl], in_=tB)

        # --- output projection + bias ---
        with (
            tc.tile_pool(name="wopool", bufs=1) as wopool,
            tc.tile_pool(name="opool", bufs=2, space="PSUM") as opool,
            tc.tile_pool(name="ospool", bufs=3) as ospool,
        ):
            wo = wopool.tile([128, DC, D], F32R)
            for k in range(DC):
                nc.sync.dma_start(out=wo[:, k, :], in_=woT[128 * k:128 * (k + 1), :])
            for i in range(SQL // 128):
                isl = slice(128 * i, 128 * (i + 1))
                po = opool.tile([128, D], F32)
                for lo, hi in ((0, 512), (512, D)):
                    for r in range(NPAIR):
                        nc.tensor.matmul(
                            po[:, lo:hi], lhsT=_r(ctx6[:, r, isl]),
                            rhs=_r(wo[:, r, lo:hi]),
                            start=(r == 0), stop=(r == NPAIR - 1))
                osb = ospool.tile([128, D], F32)
                nc.vector.tensor_add(osb, po, bo_sb)
                nc.sync.dma_start(out=out_d[isl, :], in_=osb)

    nc.compile()
    return nc


def get_nc():
    global _CACHED_NC
    if _CACHED_NC is None:
        _CACHED_NC = build_nc()
    return _CACHED_NC


def make_core_inputs(x, wq, wk, wv, wo, bo):
    """Host-side shard prep: slices/transposes only, no arithmetic."""
    wqT = np.ascontiguousarray(wq.T)
    wkT = np.ascontiguousarray(wk.T)
    wvT = np.ascontiguousarray(wv.T)
    woT = np.ascontiguousarray(wo.T)
    bo_in = np.ascontiguousarray(bo.reshape(1, D))

    p_idx = np.arange(128)[:, None]
    u_idx = np.arange(256)[None, :]
    mask_by_half = []
    for half in range(2):
        m = np.zeros((4, 128, 256), np.float32)
        for mi in range(4):
            m[mi] = (p_idx <= 2 * u_idx + half - 128 * mi).astype(np.float32)
        mask_by_half.append(m)

    in_maps = []
    for c in range(N_CORES):
        b, half = c // 2, c % 2
        xT_b = np.ascontiguousarray(x[b].T)
        in_maps.append({
            "xT": xT_b,
            "xTq": np.ascontiguousarray(xT_b[:, half::2]),
            "wqT": wqT, "wkT": wkT, "wvT": wvT, "woT": woT,
            "masks": mask_by_half[half],
            "bo": bo_in,
        })
    return in_maps


def kernel(x, wq, wk, wv, wo, bo):
    global LAST_RESULT
    x = np.asarray(x, np.float32)
    in_maps = make_core_inputs(
        x, np.asarray(wq, np.float32), np.asarray(wk, np.float32),
        np.asarray(wv, np.float32), np.asarray(wo, np.float32),
        np.asarray(bo, np.float32))

    nc = get_nc()
    trace = bool(int(os.environ.get("KERNEL_TRACE", "0")))
    kwargs = {}
    if trace:
        kwargs.update(trace=True, trace_cores=[0, 1],
                      tmpdir=os.environ.get("KERNEL_TRACE_DIR") or None)
    res = run_bass_kernel_spmd(nc, in_maps, list(range(N_CORES)), **kwargs)
    LAST_RESULT = res

    out = np.empty((B, S, D), np.float32)
    for c in range(N_CORES):
        b, half = c // 2, c % 2
        out[b, half::2, :] = res.results[c]["out"]
    return out


# revision 8
# speedup vs baseline: 1.2475x; 1.2475x over previous
"""Trainium2 Bass kernel for 12-head causal MHA (B=4, S=2048, D=768).

Sharding: 8 cores, core c -> (batch c//2, query-row parity c%2).
Each core computes the full attention output for query rows
g = 2*t + parity of its batch (1024 rows), which makes the causal loop
structure identical on every core (single SPMD Bass program) and the
gather a pure row-interleave.

Layout is fully transposed so every matmul contracts along partitions:
  qT/kT: [head_dim, seq]  scoresT: [sk, sq]  ctxT': [hd+1, sq]
The softmax row-sum is fused into the ctx matmul via a ones column
appended to V (M=65).  Softmax skips max-subtraction (scores/8 are
bounded by ~2 for this distribution, exp is safe).
"""

import os
import sys
from contextlib import ExitStack

import numpy as np

os.environ.setdefault("MYCRO_LOCAL_CACHE", "1")

for _p in ("/root/.axon_site/_ro/trn_rl_repo", "/opt/trn_rl_repo"):
    # later inserts win: prefer /opt (writable sibling modules, e.g.
    # antenv.axon_hooks) over the read-only mirror
    if os.path.isdir(_p) and _p not in sys.path:
        sys.path.insert(0, _p)

import concourse.bass as bass  # noqa: E402
import concourse.tile as tile  # noqa: E402
from concourse import bacc, mybir  # noqa: E402
from concourse.bass_utils import run_bass_kernel_spmd  # noqa: E402

B, S, D, H, HD = 4, 2048, 768, 12, 64
NPAIR = H // 2          # 6 head pairs (2 heads packed per 128 partitions)
SQL = S // 2            # 1024 local query rows per core
JB = SQL // 256         # 4 local 256-col blocks
KC = S // 128           # 16 key chunks
DC = D // 128           # 6 contraction chunks for the projections
N_CORES = 8

F32 = mybir.dt.float32
F32R = mybir.dt.float32r
BF16 = mybir.dt.bfloat16
EXP = mybir.ActivationFunctionType.Exp

LAST_RESULT = None  # BassKernelResults of the most recent run (for test.py)

_CACHED_NC = None


def _r(ap):
    """f32r matmul operand (tiles on these paths are float32r-typed)."""
    return ap


def build_nc():
    nc = bacc.Bacc("TRN2", target_bir_lowering=False)

    xT = nc.dram_tensor("xT", [D, S], BF16, kind="ExternalInput")
    xTq = nc.dram_tensor("xTq", [D, SQL], BF16, kind="ExternalInput")
    wqT = nc.dram_tensor("wqT", [D, D], BF16, kind="ExternalInput")
    wkT = nc.dram_tensor("wkT", [D, D], BF16, kind="ExternalInput")
    wvT = nc.dram_tensor("wvT", [D, D], BF16, kind="ExternalInput")
    woT = nc.dram_tensor("woT", [D, D], BF16, kind="ExternalInput")
    masks = nc.dram_tensor("masks", [4, 128, 256], BF16, kind="ExternalInput")
    bo_d = nc.dram_tensor("bo", [1, D], F32, kind="ExternalInput")
    out_d = nc.dram_tensor("out", [SQL, D], F32, kind="ExternalOutput")

    with tile.TileContext(nc) as tc, ExitStack() as ctx:
        pers = ctx.enter_context(tc.tile_pool(name="pers", bufs=1))
        kT6 = pers.tile([128, NPAIR, S], BF16)          # kT, pair-stacked
        v3 = pers.tile([128, KC, H, HD + 1], BF16)      # v (+ones col) per chunk
        qT6 = pers.tile([128, NPAIR, SQL], BF16)
        ctx6 = pers.tile([128, NPAIR, SQL], BF16)       # normalized ctxT
        ones_sb = pers.tile([65, 128], F32)
        mask_sb = pers.tile([128, 4, 256], BF16)
        bo_sb = pers.tile([128, D], F32)

        nc.vector.memset(ones_sb, 1.0)
        nc.vector.memset(v3[:, :, :, HD], 1.0)         # ones cols, stride 65
        for mi in range(4):
            nc.sync.dma_start(out=mask_sb[:, mi, :], in_=masks[mi])

        # --- broadcast bo across partitions once (rank-1 matmul trick) ---
        with (
            tc.tile_pool(name="pre_s", bufs=1) as pre_s,
            tc.tile_pool(name="pre_p", bufs=1, space="PSUM") as pre_p,
        ):
            bo_row = pre_s.tile([1, D], F32)
            nc.sync.dma_start(out=bo_row, in_=bo_d[:])
            pbo = pre_p.tile([128, D], F32)
            for lo, hi in ((0, 512), (512, D)):
                nc.tensor.matmul(pbo[:, lo:hi], lhsT=ones_sb[0:1, :],
                                 rhs=bo_row[0:1, lo:hi], start=True, stop=True)
            nc.vector.tensor_copy(bo_sb, pbo)

        # --- projections: K, V, then Q (weights staged one at a time) ---
        with (
            tc.tile_pool(name="wstage", bufs=1) as wpool,
            tc.tile_pool(name="xstage", bufs=2) as xpool,
            tc.tile_pool(name="pproj", bufs=2, space="PSUM") as ppool,
        ):
            # K projection: kT6[:, r, s] for all 2048 keys
            wk = wpool.tile([128, DC, D], BF16, tag="w")
            for k in range(DC):
                nc.sync.dma_start(out=wk[:, k, :], in_=wkT[128 * k:128 * (k + 1), :])
            for sb in range(S // 256):
                xk = xpool.tile([128, DC, 256], BF16, tag="x")
                for k in range(DC):
                    nc.sync.dma_start(
                        out=xk[:, k, :],
                        in_=xT[128 * k:128 * (k + 1), 256 * sb:256 * (sb + 1)])
                for r in range(NPAIR):
                    ps = ppool.tile([128, 256], F32, tag="pk")
                    for k in range(DC):
                        nc.tensor.matmul(
                            ps, lhsT=_r(wk[:, k, 128 * r:128 * (r + 1)]),
                            rhs=_r(xk[:, k, :]),
                            start=(k == 0), stop=(k == DC - 1))
                    nc.vector.tensor_copy(kT6[:, r, 256 * sb:256 * (sb + 1)], ps)

            # V projection: v3[:, a, h, 0:64] per 128-key chunk a
            wv = wpool.tile([128, DC, D], BF16, tag="w")
            for k in range(DC):
                nc.sync.dma_start(out=wv[:, k, :], in_=wvT[128 * k:128 * (k + 1), :])
            for a in range(KC):
                xa = xpool.tile([128, DC, 128], BF16, tag="x")
                for k in range(DC):
                    nc.sync.dma_start(
                        out=xa[:, k, :],
                        in_=xT[128 * k:128 * (k + 1), 128 * a:128 * (a + 1)])
                ps = ppool.tile([128, D], F32, tag="pv")
                for lo, hi in ((0, 512), (512, D)):
                    for k in range(DC):
                        nc.tensor.matmul(
                            ps[:, lo:hi], lhsT=_r(xa[:, k, :]),
                            rhs=_r(wv[:, k, lo:hi]),
                            start=(k == 0), stop=(k == DC - 1))
                nc.vector.tensor_copy(
                    v3[:, a, :, 0:HD],
                    ps.rearrange("p (h e) -> p h e", e=HD))

            # Q projection (only this core's 1024 query rows)
            wq = wpool.tile([128, DC, D], BF16, tag="w")
            for k in range(DC):
                nc.sync.dma_start(out=wq[:, k, :], in_=wqT[128 * k:128 * (k + 1), :])
            for j in range(JB):
                xq = xpool.tile([128, DC, 256], BF16, tag="x")
                for k in range(DC):
                    nc.sync.dma_start(
                        out=xq[:, k, :],
                        in_=xTq[128 * k:128 * (k + 1), 256 * j:256 * (j + 1)])
                for r in range(NPAIR):
                    ps = ppool.tile([128, 256], F32, tag="pk")
                    for k in range(DC):
                        nc.tensor.matmul(
                            ps, lhsT=_r(wq[:, k, 128 * r:128 * (r + 1)]),
                            rhs=_r(xq[:, k, :]),
                            start=(k == 0), stop=(k == DC - 1))
                    nc.vector.tensor_copy(qT6[:, r, 256 * j:256 * (j + 1)], ps)

        # --- attention ---
        with (
            tc.tile_pool(name="spool", bufs=5, space="PSUM") as spool,
            tc.tile_pool(name="cpool", bufs=2, space="PSUM") as cpool,
            tc.tile_pool(name="bpool", bufs=1, space="PSUM") as bpool,
            tc.tile_pool(name="epool", bufs=3) as epool,
            tc.tile_pool(name="rpool", bufs=2) as rpool,
        ):
            for r in range(NPAIR):
                for j in range(JB):
                    nch = 4 * j + 4
                    cA = cpool.tile([65, 256], F32, tag="c")
                    cB = cpool.tile([65, 256], F32, tag="c")
                    jsl = slice(256 * j, 256 * (j + 1))
                    for a in range(nch):
                        asl = slice(128 * a, 128 * (a + 1))
                        mi = a - 4 * j
                        # cols [0, z) of this (chunk, block) site are fully
                        # masked; skip them in scores/exp/ctx via slicing
                        z = 64 * mi if mi > 0 else 0
                        w = 256 - z
                        sA = spool.tile([128, 256], F32, tag="s")
                        sB = spool.tile([128, 256], F32, tag="s")
                        nc.tensor.matmul(
                            sA[:, z:256], lhsT=_r(kT6[0:64, r, asl]),
                            rhs=_r(qT6[0:64, r, 256 * j + z:256 * (j + 1)]),
                            start=True, stop=True, tile_position=(0, 0))
                        nc.tensor.matmul(
                            sB[:, z:256], lhsT=_r(kT6[64:128, r, asl]),
                            rhs=_r(qT6[64:128, r, 256 * j + z:256 * (j + 1)]),
                            start=True, stop=True, tile_position=(64, 0))
                        e = epool.tile([128, 512], BF16, tag="e")
                        nc.scalar.activation(e[:, z:256], sA[:, z:256],
                                             EXP, scale=0.125)
                        nc.scalar.activation(e[:, 256 + z:512], sB[:, z:256],
                                             EXP, scale=0.125)
                        if mi >= 0:
                            # only the 64-col mixed strip needs masking
                            ms = slice(64 * mi, 64 * mi + 64)
                            nc.vector.tensor_mul(
                                e[:, ms], e[:, ms], mask_sb[:, mi, ms])
                            ms2 = slice(256 + 64 * mi, 256 + 64 * mi + 64)
                            nc.vector.tensor_mul(
                                e[:, ms2], e[:, ms2], mask_sb[:, mi, ms])
                        nc.tensor.matmul(
                            cA[:, z:256], lhsT=_r(v3[:, a, 2 * r, :]),
                            rhs=_r(e[:, z:256]),
                            start=(a == 0), stop=(a == nch - 1))
                        nc.tensor.matmul(
                            cB[:, z:256], lhsT=_r(v3[:, a, 2 * r + 1, :]),
                            rhs=_r(e[:, 256 + z:512]),
                            start=(a == 0), stop=(a == nch - 1))
                    # normalize: recip of fused row-sums, broadcast via PE
                    rr = rpool.tile([65, 512], F32, tag="rr")
                    nc.vector.reciprocal(rr[64:65, 0:256], cA[64:65, :])
                    nc.vector.reciprocal(rr[64:65, 256:512], cB[64:65, :])
                    pb = bpool.tile([128, 512], F32, tag="b")
                    nc.tensor.matmul(pb, lhsT=ones_sb[64:65, :],
                                     rhs=rr[64:65, :], start=True, stop=True)
                    pb_sb = rpool.tile([128, 512], F32, tag="pbs")
                    nc.vector.tensor_copy(pb_sb, pb)
                    nc.vector.tensor_mul(ctx6[0:64, r, jsl], cA[0:64, :],
                                         pb_sb[0:64, 0:256])
                    tB = rpool.tile([64, 256], BF16, tag="tB")
                    nc.vector.tensor_mul(tB, cB[0:64, :], pb_sb[0:64, 256:512])
                    # head B lands on partitions 64-127: remap via SBUF DMA
                    nc.sync.dma_start(out=ctx6[64:128, r, jsl], in_=tB)

        # --- output projection + bias ---
        with (
            tc.tile_pool(name="wopool", bufs=1) as wopool,
            tc.tile_pool(name="opool", bufs=2, space="PSUM") as opool,
            tc.tile_pool(name="ospool", bufs=3) as ospool,
        ):
            wo = wopool.tile([128, DC, D], BF16)
            for k in range(DC):
                nc.sync.dma_start(out=wo[:, k, :], in_=woT[128 * k:128 * (k + 1), :])
            for i in range(SQL // 128):
                isl = slice(128 * i, 128 * (i + 1))
                po = opool.tile([128, D], F32)
                for lo, hi in ((0, 512), (512, D)):
                    for r in range(NPAIR):
                        nc.tensor.matmul(
                            po[:, lo:hi], lhsT=_r(ctx6[:, r, isl]),
                            rhs=_r(wo[:, r, lo:hi]),
                            start=(r == 0), stop=(r == NPAIR - 1))
                osb = ospool.tile([128, D], F32)
                nc.vector.tensor_add(osb, po, bo_sb)
                nc.sync.dma_start(out=out_d[isl, :], in_=osb)

    nc.compile()
    return nc


def get_nc():
    global _CACHED_NC
    if _CACHED_NC is None:
        _CACHED_NC = build_nc()
    return _CACHED_NC


def make_core_inputs(x, wq, wk, wv, wo, bo):
    """Host-side shard prep: slices/transposes/dtype rounding only."""
    import ml_dtypes
    bf16 = ml_dtypes.bfloat16
    wqT = np.ascontiguousarray(wq.T.astype(bf16))
    wkT = np.ascontiguousarray(wk.T.astype(bf16))
    wvT = np.ascontiguousarray(wv.T.astype(bf16))
    woT = np.ascontiguousarray(wo.T.astype(bf16))
    bo_in = np.ascontiguousarray(bo.reshape(1, D))

    p_idx = np.arange(128)[:, None]
    u_idx = np.arange(256)[None, :]
    mask_by_half = []
    for half in range(2):
        m = np.zeros((4, 128, 256), ml_dtypes.bfloat16)
        for mi in range(4):
            m[mi] = (p_idx <= 2 * u_idx + half - 128 * mi)
        mask_by_half.append(m)

    in_maps = []
    for c in range(N_CORES):
        b, half = c // 2, c % 2
        xT_b = np.ascontiguousarray(x[b].T.astype(bf16))
        in_maps.append({
            "xT": xT_b,
            "xTq": np.ascontiguousarray(xT_b[:, half::2]),
            "wqT": wqT, "wkT": wkT, "wvT": wvT, "woT": woT,
            "masks": mask_by_half[half],
            "bo": bo_in,
        })
    return in_maps


def kernel(x, wq, wk, wv, wo, bo):
    global LAST_RESULT
    x = np.asarray(x, np.float32)
    in_maps = make_core_inputs(
        x, np.asarray(wq, np.float32), np.asarray(wk, np.float32),
        np.asarray(wv, np.float32), np.asarray(wo, np.float32),
        np.asarray(bo, np.float32))

    nc = get_nc()
    trace = bool(int(os.environ.get("KERNEL_TRACE", "0")))
    kwargs = {}
    if trace:
        kwargs.update(trace=True, trace_cores=[0, 1],
                      tmpdir=os.environ.get("KERNEL_TRACE_DIR") or None)
    res = run_bass_kernel_spmd(nc, in_maps, list(range(N_CORES)), **kwargs)
    LAST_RESULT = res

    out = np.empty((B, S, D), np.float32)
    for c in range(N_CORES):
        b, half = c // 2, c % 2
        out[b, half::2, :] = res.results[c]["out"]
    return out


# revision 15
# speedup vs baseline: 1.3260x; 1.0629x over previous
"""Trainium2 Bass kernel for 12-head causal MHA (B=4, S=2048, D=768).

Sharding: 8 cores, core c -> (batch c//2, query-row parity c%2).
Each core computes the full attention output for query rows
g = 2*t + parity of its batch (1024 rows), which makes the causal loop
structure identical on every core (single SPMD Bass program) and the
gather a pure row-interleave.

Layout is fully transposed so every matmul contracts along partitions:
  qT/kT: [head_dim, seq]  scoresT: [sk, sq]  ctxT': [hd+1, sq]
The softmax row-sum is fused into the ctx matmul via a ones column
appended to V (M=65).  Softmax skips max-subtraction (scores/8 are
bounded by ~2 for this distribution, exp is safe).
"""

import os
import sys
from contextlib import ExitStack

import numpy as np

os.environ.setdefault("MYCRO_LOCAL_CACHE", "1")

for _p in ("/root/.axon_site/_ro/trn_rl_repo", "/opt/trn_rl_repo"):
    # later inserts win: prefer /opt (writable sibling modules, e.g.
    # antenv.axon_hooks) over the read-only mirror
    if os.path.isdir(_p) and _p not in sys.path:
        sys.path.insert(0, _p)

import concourse.bass as bass  # noqa: E402
import concourse.tile as tile  # noqa: E402
from concourse import bacc, mybir  # noqa: E402
from concourse.bass_utils import run_bass_kernel_spmd  # noqa: E402

B, S, D, H, HD = 4, 2048, 768, 12, 64
NPAIR = H // 2          # 6 head pairs (2 heads packed per 128 partitions)
SQL = S // 2            # 1024 local query rows per core
JB = SQL // 256         # 4 local 256-col blocks
KC = S // 128           # 16 key chunks
DC = D // 128           # 6 contraction chunks for the projections
N_CORES = 8

F32 = mybir.dt.float32
F32R = mybir.dt.float32r
BF16 = mybir.dt.bfloat16
EXP = mybir.ActivationFunctionType.Exp

LAST_RESULT = None  # BassKernelResults of the most recent run (for test.py)

_CACHED_NC = None


def _r(ap):
    """f32r matmul operand (tiles on these paths are float32r-typed)."""
    return ap


def build_nc():
    nc = bacc.Bacc("TRN2", target_bir_lowering=False)

    xT = nc.dram_tensor("xT", [D, S], BF16, kind="ExternalInput")
    xTq = nc.dram_tensor("xTq", [D, SQL], BF16, kind="ExternalInput")
    wqT = nc.dram_tensor("wqT", [D, D], BF16, kind="ExternalInput")
    wkT = nc.dram_tensor("wkT", [D, D], BF16, kind="ExternalInput")
    wvT = nc.dram_tensor("wvT", [D, D], BF16, kind="ExternalInput")
    woT = nc.dram_tensor("woT", [D, D], BF16, kind="ExternalInput")
    masks = nc.dram_tensor("masks", [4, 128, 256], BF16, kind="ExternalInput")
    bo_d = nc.dram_tensor("bo", [1, D], F32, kind="ExternalInput")
    out_d = nc.dram_tensor("out", [SQL, D], F32, kind="ExternalOutput")

    with tile.TileContext(nc) as tc, ExitStack() as ctx:
        pers = ctx.enter_context(tc.tile_pool(name="pers", bufs=1))
        kT6 = pers.tile([128, NPAIR, S], BF16)          # kT, pair-stacked
        v3 = pers.tile([128, KC, H, HD + 1], BF16)      # v (+ones col) per chunk
        qT6 = pers.tile([128, NPAIR, SQL], BF16)
        ctx6 = pers.tile([128, NPAIR, SQL], BF16)       # normalized ctxT
        ones_sb = pers.tile([65, 128], F32)
        mask_sb = pers.tile([128, 4, 256], BF16)
        bo_sb = pers.tile([128, D], F32)

        nc.vector.memset(ones_sb, 1.0)
        nc.vector.memset(v3[:, :, :, HD], 1.0)         # ones cols, stride 65
        for mi in range(4):
            nc.sync.dma_start(out=mask_sb[:, mi, :], in_=masks[mi])

        # --- broadcast bo across partitions once (rank-1 matmul trick) ---
        with (
            tc.tile_pool(name="pre_s", bufs=1) as pre_s,
            tc.tile_pool(name="pre_p", bufs=1, space="PSUM") as pre_p,
        ):
            bo_row = pre_s.tile([1, D], F32)
            nc.sync.dma_start(out=bo_row, in_=bo_d[:])
            pbo = pre_p.tile([128, D], F32)
            for lo, hi in ((0, 512), (512, D)):
                nc.tensor.matmul(pbo[:, lo:hi], lhsT=ones_sb[0:1, :],
                                 rhs=bo_row[0:1, lo:hi], start=True, stop=True)
            nc.vector.tensor_copy(bo_sb, pbo)

        # --- projections: K, V, then Q (weights staged one at a time) ---
        with (
            tc.tile_pool(name="wstage", bufs=3) as wpool,
            tc.tile_pool(name="xstage", bufs=3) as xpool,
            tc.tile_pool(name="pproj", bufs=3, space="PSUM") as ppool,
        ):
            # K projection: kT6[:, r, s] for all 2048 keys
            wk = wpool.tile([128, DC, D], BF16, tag="w")
            for k in range(DC):
                nc.sync.dma_start(out=wk[:, k, :], in_=wkT[128 * k:128 * (k + 1), :])
            for sb in range(S // 512):
                xk = xpool.tile([128, DC, 512], BF16, tag="x")
                for k in range(DC):
                    nc.sync.dma_start(
                        out=xk[:, k, :],
                        in_=xT[128 * k:128 * (k + 1), 512 * sb:512 * (sb + 1)])
                for r in range(NPAIR):
                    ps = ppool.tile([128, 512], F32, tag="pk")
                    for k in range(DC):
                        nc.tensor.matmul(
                            ps, lhsT=_r(wk[:, k, 128 * r:128 * (r + 1)]),
                            rhs=_r(xk[:, k, :]),
                            start=(k == 0), stop=(k == DC - 1))
                    nc.vector.tensor_copy(kT6[:, r, 512 * sb:512 * (sb + 1)], ps)

            # V projection: v3[:, a, h, 0:64] per 128-key chunk a
            wv = wpool.tile([128, DC, D], BF16, tag="w")
            for k in range(DC):
                nc.sync.dma_start(out=wv[:, k, :], in_=wvT[128 * k:128 * (k + 1), :])
            for a in range(KC):
                xa = xpool.tile([128, DC, 128], BF16, tag="x")
                for k in range(DC):
                    nc.sync.dma_start(
                        out=xa[:, k, :],
                        in_=xT[128 * k:128 * (k + 1), 128 * a:128 * (a + 1)])
                ps = ppool.tile([128, D], F32, tag="pk")
                for lo, hi in ((0, 512), (512, D)):
                    for k in range(DC):
                        nc.tensor.matmul(
                            ps[:, lo:hi], lhsT=_r(xa[:, k, :]),
                            rhs=_r(wv[:, k, lo:hi]),
                            start=(k == 0), stop=(k == DC - 1))
                nc.vector.tensor_copy(
                    v3[:, a, :, 0:HD],
                    ps.rearrange("p (h e) -> p h e", e=HD))

            # Q projection (only this core's 1024 query rows)
            wq = wpool.tile([128, DC, D], BF16, tag="w")
            for k in range(DC):
                nc.sync.dma_start(out=wq[:, k, :], in_=wqT[128 * k:128 * (k + 1), :])
            for j2 in range(SQL // 512):
                xq = xpool.tile([128, DC, 512], BF16, tag="x")
                for k in range(DC):
                    nc.sync.dma_start(
                        out=xq[:, k, :],
                        in_=xTq[128 * k:128 * (k + 1), 512 * j2:512 * (j2 + 1)])
                for r in range(NPAIR):
                    ps = ppool.tile([128, 512], F32, tag="pk")
                    for k in range(DC):
                        nc.tensor.matmul(
                            ps, lhsT=_r(wq[:, k, 128 * r:128 * (r + 1)]),
                            rhs=_r(xq[:, k, :]),
                            start=(k == 0), stop=(k == DC - 1))
                    nc.vector.tensor_copy(qT6[:, r, 512 * j2:512 * (j2 + 1)], ps)

        # --- attention ---
        with (
            tc.tile_pool(name="spool", bufs=2, space="PSUM") as spool,
            tc.tile_pool(name="cpool", bufs=3, space="PSUM") as cpool,
            tc.tile_pool(name="bpool", bufs=1, space="PSUM") as bpool,
            tc.tile_pool(name="epool", bufs=3) as epool,
            tc.tile_pool(name="rpool", bufs=2) as rpool,
        ):
            for r in range(NPAIR):
                for j in range(JB):
                    nch = 4 * j + 4
                    cA = cpool.tile([65, 256], F32, tag="c")
                    cB = cpool.tile([65, 256], F32, tag="c")
                    jsl = slice(256 * j, 256 * (j + 1))
                    for a in range(nch):
                        asl = slice(128 * a, 128 * (a + 1))
                        mi = a - 4 * j
                        z = 64 * mi if mi > 0 else 0
                        sA = spool.tile([128, 256], F32, tag="sA")
                        sB = spool.tile([128, 256], F32, tag="sB")
                        nc.tensor.matmul(
                            sA[:, z:256], lhsT=_r(kT6[0:64, r, asl]),
                            rhs=_r(qT6[0:64, r, 256 * j + z:256 * (j + 1)]),
                            start=True, stop=True, tile_position=(0, 0))
                        nc.tensor.matmul(
                            sB[:, z:256], lhsT=_r(kT6[64:128, r, asl]),
                            rhs=_r(qT6[64:128, r, 256 * j + z:256 * (j + 1)]),
                            start=True, stop=True, tile_position=(64, 0))
                        e = epool.tile([128, 512], BF16, tag="e")
                        nc.scalar.activation(e[:, z:256], sA[:, z:256],
                                             EXP, scale=0.125)
                        nc.scalar.activation(e[:, 256 + z:512], sB[:, z:256],
                                             EXP, scale=0.125)
                        if mi >= 0:
                            ms = slice(64 * mi, 64 * mi + 64)
                            nc.vector.tensor_mul(
                                e[:, ms], e[:, ms], mask_sb[:, mi, ms])
                            ms2 = slice(256 + 64 * mi, 256 + 64 * mi + 64)
                            nc.vector.tensor_mul(
                                e[:, ms2], e[:, ms2], mask_sb[:, mi, ms])
                        nc.tensor.matmul(
                            cA[:, z:256], lhsT=_r(v3[:, a, 2 * r, :]),
                            rhs=_r(e[:, z:256]),
                            start=(a == 0), stop=(a == nch - 1))
                        nc.tensor.matmul(
                            cB[:, z:256], lhsT=_r(v3[:, a, 2 * r + 1, :]),
                            rhs=_r(e[:, 256 + z:512]),
                            start=(a == 0), stop=(a == nch - 1))
                    # normalize: recip of fused row-sums, broadcast via PE
                    rr = rpool.tile([65, 512], F32, tag="rr")
                    nc.vector.reciprocal(rr[64:65, 0:256], cA[64:65, :])
                    nc.vector.reciprocal(rr[64:65, 256:512], cB[64:65, :])
                    pb = bpool.tile([128, 512], F32, tag="b")
                    nc.tensor.matmul(pb, lhsT=ones_sb[64:65, :],
                                     rhs=rr[64:65, :], start=True, stop=True)
                    pb_sb = rpool.tile([128, 512], F32, tag="pbs")
                    nc.vector.tensor_copy(pb_sb, pb)
                    nc.vector.tensor_mul(ctx6[0:64, r, jsl], cA[0:64, :],
                                         pb_sb[0:64, 0:256])
                    tB = rpool.tile([64, 256], BF16, tag="tB")
                    nc.vector.tensor_mul(tB, cB[0:64, :], pb_sb[0:64, 256:512])
                    # head B lands on partitions 64-127: remap via SBUF DMA
                    nc.sync.dma_start(out=ctx6[64:128, r, jsl], in_=tB)

        # --- output projection + bias ---
        with (
            tc.tile_pool(name="wopool", bufs=1) as wopool,
            tc.tile_pool(name="opool", bufs=2, space="PSUM") as opool,
            tc.tile_pool(name="ospool", bufs=3) as ospool,
        ):
            wo = wopool.tile([128, DC, D], BF16)
            for k in range(DC):
                nc.sync.dma_start(out=wo[:, k, :], in_=woT[128 * k:128 * (k + 1), :])
            for i in range(SQL // 128):
                isl = slice(128 * i, 128 * (i + 1))
                po = opool.tile([128, D], F32)
                for lo, hi in ((0, 512), (512, D)):
                    for r in range(NPAIR):
                        nc.tensor.matmul(
                            po[:, lo:hi], lhsT=_r(ctx6[:, r, isl]),
                            rhs=_r(wo[:, r, lo:hi]),
                            start=(r == 0), stop=(r == NPAIR - 1))
                osb = ospool.tile([128, D], F32)
                nc.vector.tensor_add(osb, po, bo_sb)
                nc.sync.dma_start(out=out_d[isl, :], in_=osb)

    nc.compile()
    return nc


def get_nc():
    global _CACHED_NC
    if _CACHED_NC is None:
        _CACHED_NC = build_nc()
    return _CACHED_NC


def make_core_inputs(x, wq, wk, wv, wo, bo):
    """Host-side shard prep: slices/transposes/dtype rounding only."""
    import ml_dtypes
    bf16 = ml_dtypes.bfloat16
    wqT = np.ascontiguousarray(wq.T.astype(bf16))
    wkT = np.ascontiguousarray(wk.T.astype(bf16))
    wvT = np.ascontiguousarray(wv.T.astype(bf16))
    woT = np.ascontiguousarray(wo.T.astype(bf16))
    bo_in = np.ascontiguousarray(bo.reshape(1, D))

    p_idx = np.arange(128)[:, None]
    u_idx = np.arange(256)[None, :]
    mask_by_half = []
    for half in range(2):
        m = np.zeros((4, 128, 256), ml_dtypes.bfloat16)
        for mi in range(4):
            m[mi] = (p_idx <= 2 * u_idx + half - 128 * mi)
        mask_by_half.append(m)

    in_maps = []
    for c in range(N_CORES):
        b, half = c // 2, c % 2
        xT_b = np.ascontiguousarray(x[b].T.astype(bf16))
        in_maps.append({
            "xT": xT_b,
            "xTq": np.ascontiguousarray(xT_b[:, half::2]),
            "wqT": wqT, "wkT": wkT, "wvT": wvT, "woT": woT,
            "masks": mask_by_half[half],
            "bo": bo_in,
        })
    return in_maps


def kernel(x, wq, wk, wv, wo, bo):
    global LAST_RESULT
    x = np.asarray(x, np.float32)
    in_maps = make_core_inputs(
        x, np.asarray(wq, np.float32), np.asarray(wk, np.float32),
        np.asarray(wv, np.float32), np.asarray(wo, np.float32),
        np.asarray(bo, np.float32))

    nc = get_nc()
    trace = bool(int(os.environ.get("KERNEL_TRACE", "0")))
    kwargs = {}
    if trace:
        kwargs.update(trace=True, trace_cores=[0, 1],
                      tmpdir=os.environ.get("KERNEL_TRACE_DIR") or None)
    res = run_bass_kernel_spmd(nc, in_maps, list(range(N_CORES)), **kwargs)
    LAST_RESULT = res

    out = np.empty((B, S, D), np.float32)
    for c in range(N_CORES):
        b, half = c // 2, c % 2
        out[b, half::2, :] = res.results[c]["out"]
    return out
